# revision 33
# baseline (speedup 1.0000x reference)
"""Trainium2 Bass kernel for nn_Attention_v4 — v3 (bf16, pipelined).

Sharding: 4 groups x 7 (b,r)-units data-parallel, x 2 head-halves tensor
parallel (8 cores). Host sums the two head-half c_proj partials + b_proj.

v3 structure:
  - all-bf16 matmuls (fp8 fails the 2e-2 gate: one fp8-quantized tensor
    alone contributes ~3e-3 abs error; bf16 is ~10x cleaner)
  - scores row-tiled: per head pair, K=64 matmuls at tile_position (0,0)
    and (64,0) run concurrently on the PE's 32-row sub-arrays
  - cross-unit software pipeline: window u emits [dma x(u+1)] then
    interleaves A-steps(u) with C(u-1) and Q(u+1) tasks, so the PE always
    has independent work while ACT streams the exps and DVE normalizes
  - paired exps: one ACT op per head-pair covers two [128,512] score tiles
  - out copy on ACT (Copy, scale=1/SP) -> bf16 DMA
Layouts:
  qk_sb [128, 8, tok] bf16 - ct 0-3 q head-pairs, 4-7 k head-pairs,
                             head lh at rows (lh%2)*64
  v_sb  [tok, tt(4), lh*66] - cols 0:64 v, 64 rider 1.0, 65 pad
  po    [66, tok] psum      - row 64 = softmax denominator
"""

import numpy as np
import ml_dtypes

B, N, A, DIM, H, D = 2, 512, 14, 1024, 16, 64
HL = 8
UNITS = 7
NCORES = 8
SCALE = np.float32(1.0 / np.sqrt(np.sqrt(D)))
VW = D + 2
S1 = np.float32(1.0)
SV = np.float32(1.0)
SP = np.float32(1.0)

_CACHE = {}


def _build_nc(units=UNITS, repeat=1, phases="QAC", qk_bias=False):
    import concourse.bacc as bacc
    import concourse.tile as tile
    from concourse import mybir
    from concourse.bass import ts

    f32, f32r = mybir.dt.float32, mybir.dt.float32r
    bf16 = mybir.dt.bfloat16
    AF = mybir.ActivationFunctionType

    nc = bacc.Bacc("TRN2", target_bir_lowering=False, debug=False,
                   num_devices=NCORES)
    xT = nc.dram_tensor("xT", [units, DIM, N], bf16, kind="ExternalInput")
    wqkv = nc.dram_tensor("wqkv", [DIM, 1024 + HL * D], bf16,
                          kind="ExternalInput")
    bqk = nc.dram_tensor("bqk", [1024], f32, kind="ExternalInput")
    bv = nc.dram_tensor("bv", [HL * VW], f32, kind="ExternalInput")
    wproj = nc.dram_tensor("wproj", [HL * D, DIM], bf16, kind="ExternalInput")
    part = nc.dram_tensor("part", [units, N, DIM], bf16,
                          kind="ExternalOutput")

    import concourse.bass as bass

    def bcast_part(ap, p=128):
        return bass.AP(tensor=ap.tensor, offset=ap.offset,
                       ap=[[0, p]] + list(ap.ap))

    with tile.TileContext(nc) as tc:
        import contextlib
        with contextlib.ExitStack() as ctx:
            const = ctx.enter_context(tc.tile_pool(name="const", bufs=1))
            p_x = ctx.enter_context(tc.tile_pool(name="p_x", bufs=2))
            p_qk = ctx.enter_context(tc.tile_pool(name="p_qk", bufs=2))
            p_v = ctx.enter_context(tc.tile_pool(name="p_v", bufs=2))
            p_es = ctx.enter_context(tc.tile_pool(name="p_es", bufs=6))
            p_ot = ctx.enter_context(tc.tile_pool(name="p_ot", bufs=2))
            p_out = ctx.enter_context(tc.tile_pool(name="p_out", bufs=3))
            p_rc = ctx.enter_context(tc.tile_pool(name="p_rc", bufs=4))
            ps_big = ctx.enter_context(
                tc.tile_pool(name="ps_big", bufs=2, space="PSUM"))
            ps_o = ctx.enter_context(
                tc.tile_pool(name="ps_o", bufs=4, space="PSUM"))

            # ---- persistent weights ----
            wq_sb = const.tile([128, 8, 1024 + HL * D], bf16, tag="wqkv")
            _wq_r = wqkv[:].rearrange("(k p) c -> p k c", p=128)
            for k in range(8):
                nc.sync.dma_start(out=wq_sb[:, k, :], in_=_wq_r[:, k, :])
            wp_sb = const.tile([128, 4, DIM], bf16, tag="wproj")
            nc.sync.dma_start(
                out=wp_sb, in_=wproj[:].rearrange("(k p) c -> p k c", p=128))
            bqk_sb = const.tile([128, 8], f32, tag="bqk")
            nc.sync.dma_start(
                out=bqk_sb, in_=bqk[:].rearrange("(c p) -> p c", p=128))
            bv_sb = const.tile([128, HL * VW], f32, tag="bv")
            nc.sync.dma_start(out=bv_sb, in_=bcast_part(bv[:]))
            bvv = bv_sb.rearrange("p (h w) -> p h w", w=VW)

            class Unit:
                def __init__(self, u):
                    self.u = u

                def q_tasks(self):
                    U = self

                    def dma_x():
                        U.x = p_x.tile([128, 8, N], bf16, tag="x", name="x_sb")
                        nc.sync.dma_start(
                            out=U.x,
                            in_=xT[U.u].rearrange("(k p) n -> p k n", p=128))
                        U.qk = p_qk.tile([128, 8, N], bf16, tag="qk", name="qk_sb")
                        U.v = p_v.tile([128, 4, HL * VW], bf16, tag="v", name="v_sb")

                    tasks = []

                    def qk_group(cp):
                        pm = ps_big.tile([128, 2, N], f32, tag="big", name="pm")
                        for sub in range(2):
                            ct = 2 * cp + sub
                            for k in range(8):
                                nc.tensor.matmul(
                                    pm[:, sub, :], wq_sb[:, k, ts(ct, 128)],
                                    U.x[:, k, :],
                                    start=(k == 0), stop=(k == 7))
                        if qk_bias:
                            for sub in range(2):
                                ct = 2 * cp + sub
                                nc.vector.tensor_scalar_add(
                                    U.qk[:, ct, :], pm[:, sub, :],
                                    bqk_sb[:, ct:ct + 1])
                        else:
                            nc.vector.tensor_copy(
                                out=U.qk[:, 2 * cp:2 * cp + 2, :], in_=pm)

                    def v_group(p2):
                        pw = ps_big.tile([128, 2, N], f32, tag="big", name="pw")
                        vv = U.v.rearrange("p t (h w) -> p t h w", w=VW)
                        for sub in range(2):
                            tt = 2 * p2 + sub
                            pv = pw[:, sub, :]
                            for k in range(8):
                                nc.tensor.matmul(
                                    pv, U.x[:, k, ts(tt, 128)],
                                    wq_sb[:, k, 1024:1024 + HL * D],
                                    start=(k == 0), stop=(k == 7))
                            pvv = pv.rearrange("p (h d) -> p h d", d=D)
                            nc.vector.tensor_add(
                                out=vv[:, tt, :, 0:D], in0=pvv,
                                in1=bvv[:, :, 0:D])
                            nc.vector.tensor_copy(
                                out=vv[:, tt, :, D:VW], in_=bvv[:, :, D:VW])

                    for cp in range(4):
                        tasks.append(lambda cp=cp: qk_group(cp))
                    for p2 in range(2):
                        tasks.append(lambda p2=p2: v_group(p2))
                    return dma_x, tasks

                def a_init(self):
                    self.ot = p_ot.tile([128, 4, N], bf16, tag="ot", name="ot_sb")
                    self.pos = {}
                    self.ess = {}
                    self.pend = []

                def st_step(self, c, jt):
                    pst = ps_big.tile([128, 2, N], f32, tag="big", name="pst")
                    for h01 in range(2):
                        bp = h01 * 64
                        nc.tensor.matmul(
                            pst[:, h01, :],
                            self.qk[bp:bp + 64, 4 + c, ts(jt, 128)],
                            self.qk[bp:bp + 64, c, :],
                            start=True, stop=True,
                            tile_position=(bp, 0))
                    es2 = p_es.tile([128, 2, N], bf16, tag="es", name="es2")
                    nc.scalar.activation(
                        out=es2, in_=pst, func=AF.Exp,
                        scale=float(SCALE * SCALE / (S1 * S1)))
                    self.ess[(c, jt)] = es2

                def normalize(self, lh, po, rc):
                    bp = (lh % 2) * 64
                    bc = p_rc.tile([64, N], f32r, tag="bc", name="bc")
                    nc.gpsimd.partition_broadcast(bc, rc[0:1, :])
                    nc.vector.tensor_mul(
                        out=self.ot[bp:bp + 64, lh // 2, :],
                        in0=po[0:64, :], in1=bc)

                def pav_step(self, c, jt):
                    es2 = self.ess.pop((c, jt))
                    for h01 in range(2):
                        lh = 2 * c + h01
                        if jt == 0:
                            self.pos[lh] = ps_o.tile([VW, N], f32, tag="o", name="po")
                        po = self.pos[lh]
                        nc.tensor.matmul(
                            po, self.v[:, jt, lh * VW:(lh + 1) * VW],
                            es2[:, h01, :], start=(jt == 0), stop=(jt == 3))
                        if jt == 3:
                            rc = p_rc.tile([1, N], f32r, tag="rc", name="rc")
                            with nc.allow_low_precision(
                                    reason="f32r softmax recip"):
                                nc.vector.reciprocal(
                                    out=rc[0:1, :], in_=po[D:D + 1, :])
                            self.pend.append((lh, self.pos.pop(lh), rc))
                            while len(self.pend) > 2:
                                self.normalize(*self.pend.pop(0))

                def flush_norm(self):
                    while self.pend:
                        self.normalize(*self.pend.pop(0))

                def c_tasks(self):
                    U = self
                    tasks = []

                    def c_group(tt):
                        pc = ps_big.tile([128, 2, N], f32, tag="big", name="pc")
                        for eh in range(2):
                            for ct in range(4):
                                nc.tensor.matmul(
                                    pc[:, eh, :],
                                    U.ot[:, ct, ts(tt, 128)],
                                    wp_sb[:, ct, eh * 512:(eh + 1) * 512],
                                    start=(ct == 0), stop=(ct == 3))
                        o_sb = p_out.tile([128, DIM], bf16, tag="out", name="o_sb")
                        nc.scalar.activation(
                            out=o_sb, in_=pc.rearrange("p a b -> p (a b)"),
                            func=AF.Copy, scale=float(1.0 / SP))
                        # ACT DGE queue: keeps part writes off the SP queue
                        # so they never serialize behind x prefetch DMAs
                        nc.scalar.dma_start(
                            out=part[U.u, ts(tt, 128), :], in_=o_sb)

                    for tt in range(4):
                        tasks.append(lambda tt=tt: c_group(tt))
                    return tasks

            def emit_pass():
                us = [Unit(u) for u in range(units)]
                dma0, q0 = us[0].q_tasks()
                dma0()
                for t in q0:
                    t()
                prev_c = []
                steps = [(c, jt) for c in range(4) for jt in range(4)]
                LOOK = 2
                for i, U in enumerate(us):
                    U.a_init()
                    if i + 1 < len(us):
                        dma_n, q_n = us[i + 1].q_tasks()
                        dma_n()
                    else:
                        q_n = []
                    others = prev_c + q_n
                    oi = 0
                    for s in range(len(steps) + LOOK):
                        if s < len(steps):
                            U.st_step(*steps[s])
                        target = ((s + 1) * len(others)) // len(steps)
                        while oi < min(target, len(others)):
                            others[oi]()
                            oi += 1
                        if s >= LOOK:
                            U.pav_step(*steps[s - LOOK])
                    while oi < len(others):
                        others[oi]()
                        oi += 1
                    U.flush_norm()
                    prev_c = U.c_tasks()
                for t in prev_c:
                    t()

            if repeat == 1:
                emit_pass()
            else:
                with tc.For_i(0, repeat, 1):
                    emit_pass()

    nc.compile()
    return nc


def _make_runner(nc, n_cores=NCORES, donate=True):
    """Persistent jitted SPMD runner (mirrors bass2jax.run_bass_via_pjrt)."""
    import jax
    from jax.sharding import Mesh, PartitionSpec
    from jax.experimental.shard_map import shard_map
    from concourse import bass2jax
    from concourse import mybir as mb

    bass2jax.install_neuronx_cc_hook()
    pn = nc.partition_id_tensor.name if nc.partition_id_tensor else None
    in_names, out_names, out_avals, out_shapes = [], [], [], []
    for alloc in nc.m.functions[0].allocations:
        if not isinstance(alloc, mb.MemoryLocationSet):
            continue
        name = alloc.memorylocations[0].name
        if alloc.kind == "ExternalInput":
            if name != pn:
                in_names.append(name)
        elif alloc.kind == "ExternalOutput":
            shape = tuple(alloc.tensor_shape)
            dtype = mb.dt.np(alloc.dtype)
            out_names.append(name)
            out_avals.append(jax.core.ShapedArray(shape, dtype))
            out_shapes.append((shape, dtype))
    n_params = len(in_names)
    n_outs = len(out_names)
    all_in = list(in_names) + list(out_names) + ([pn] if pn else [])

    def _body(*args):
        ops = list(args)
        if pn:
            ops.append(bass2jax.partition_id_tensor())
        return tuple(bass2jax._bass_exec_p.bind(
            *ops, out_avals=tuple(out_avals), in_names=tuple(all_in),
            out_names=tuple(out_names), lowering_input_output_aliases=(),
            sim_require_finite=True, sim_require_nnan=True, nc=nc))

    devices = jax.devices()[:n_cores]
    mesh = Mesh(np.asarray(devices), ("core",))
    specs = (PartitionSpec("core"),)
    fn = jax.jit(
        shard_map(_body, mesh=mesh, in_specs=specs * (n_params + n_outs),
                  out_specs=specs * n_outs, check_rep=False),
        donate_argnums=tuple(range(n_params, n_params + n_outs)) if donate else (),
        keep_unused=True)

    def run(in_maps):
        per_core = [[np.asarray(m[name]) for name in in_names] for m in in_maps]
        concat_in = [np.concatenate([per_core[c][i] for c in range(n_cores)],
                                    axis=0) for i in range(n_params)]
        concat_zeros = [np.zeros((n_cores * s[0], *s[1:]), d)
                        for (s, d) in out_shapes]
        import jax as _jax
        out_arrs = _jax.block_until_ready(fn(*concat_in, *concat_zeros))
        return [
            {name: np.asarray(out_arrs[i]).reshape(n_cores, *out_shapes[i][0])[c]
             for i, name in enumerate(out_names)}
            for c in range(n_cores)
        ]

    run.jit_fn = fn
    run.in_names = in_names
    run.out_names = out_names
    run.out_shapes = out_shapes
    run.n_cores = n_cores
    return run


def _unit_groups():
    units = [(b, r) for b in range(B) for r in range(A)]
    return [units[g * UNITS:(g + 1) * UNITS] for g in range(4)]


def shard_inputs(x, w_qkv, b_qkv, w_proj, b_proj):
    groups = _unit_groups()
    w4 = w_qkv.reshape(DIM, H, 3, D)
    b4 = b_qkv.reshape(H, 3, D)
    in_maps = []
    for c in range(NCORES):
        g, hh = c // 2, c % 2
        heads = list(range(hh * HL, (hh + 1) * HL))
        xT = np.ascontiguousarray(
            np.stack([x[b, :, r, :].T for (b, r) in groups[g]])
        ).astype(ml_dtypes.bfloat16)
        wq = w4[:, heads, 0, :].reshape(DIM, HL * D) * S1
        wk = w4[:, heads, 1, :].reshape(DIM, HL * D) * S1
        wv = w4[:, heads, 2, :].reshape(DIM, HL * D) * SV
        wqkv_c = np.concatenate(
            [wq, wk, wv], axis=1).astype(ml_dtypes.bfloat16)
        bq = b4[heads, 0, :].reshape(HL * D) * S1
        bk = b4[heads, 1, :].reshape(HL * D) * S1
        bvv = np.concatenate(
            [b4[heads, 2, :] * SV,
             np.ones((HL, 1), np.float32),
             np.zeros((HL, 1), np.float32)], axis=1).reshape(HL * VW)
        in_maps.append({
            "xT": xT,
            "wqkv": wqkv_c,
            "bqk": np.concatenate([bq, bk]).astype(np.float32),
            "bv": bvv.astype(np.float32),
            "wproj": (w_proj[hh * HL * D:(hh + 1) * HL * D, :]
                      * (SP / SV)).astype(ml_dtypes.bfloat16),
        })
    return in_maps


def unshard(results, b_proj):
    groups = _unit_groups()
    out = np.zeros((B, N, A, DIM), np.float32)
    for g in range(4):
        s = (results[2 * g]["part"].astype(np.float32)
             + results[2 * g + 1]["part"].astype(np.float32))
        for idx, (b, r) in enumerate(groups[g]):
            out[b, :, r, :] = s[idx]
    return out + b_proj.astype(np.float32)


def get_runner(qk_bias=False):
    key = ("runner", qk_bias)
    if key not in _CACHE:
        nc = _build_nc(qk_bias=qk_bias)
        _CACHE[key] = _make_runner(nc)
    return _CACHE[key]


def kernel(x, w_qkv, b_qkv, w_proj, b_proj):
    x = np.asarray(x)
    w_qkv = np.asarray(w_qkv)
    b_qkv = np.asarray(b_qkv)
    w_proj = np.asarray(w_proj)
    b_proj = np.asarray(b_proj)
    run = get_runner(qk_bias=bool(np.any(b_qkv[:2048])))
    in_maps = shard_inputs(x, w_qkv, b_qkv, w_proj, b_proj)
    results = run(in_maps)
    return unshard(results, b_proj)
